# revision 2
# baseline (speedup 1.0000x reference)
"""Multi-head self-attention Trainium2 kernel (8 NeuronCores), v2.

Problem: B=4, S=2048, D=1024, H=8 heads (HD=128).
  qkv = x @ qkv_w.T + qkv_b ; q,k,v = split(qkv)
  q = (q @ q_w.T + q_b)  (same k, v) -> [B,H,S,HD]
  scores = q k^T * HD^-0.5, masked softmax (attn_mask==1 -> -inf), o = attn @ v
  out = o @ out_w.T + out_b

Sharding: 8 cores = 4 batches x 2 head-groups (4 heads each).
Core c: batch b = c % 4, head-group g = c // 4.

Host-side algebraic folding: per-stream projections composed with the qkv
projection into single effective weights; the v bias passes through softmax
unchanged (weights sum to 1) so it folds into the host-side output bias.

v3 speed structure vs v1:
  - q/k projections run in fp8e4 DoubleRow mode (K=2x128 per instruction,
    0.5 cyc/row): weights are scaled x64 on host to stay in fp8 normal
    range; the ACT copy out of PSUM rescales by 1/64 and adds the bias,
    writing fp8 directly.
  - scores stay bf16: fp8 DoubleRow with K=2x64 gives no HW speedup (DR
    only halves instruction count when K=256; stream rate is unchanged),
    and DVE/ACT ops touching PSUM at 512-col granularity halved the
    attention-phase matmul cadence in v2 — the attention inner loop is
    the v1 shape (sT [128,1024] PSUM -> ACT exp -> DVE mask-mul in SBUF),
    with the keep mask stored fp8e4 {1,0} (4MB DMA instead of 8MB).
  - v / o / out-projection matmuls stay bf16: fp8 there provably exceeds
    the error budget (random-sign dot products keep elementwise rel err).
  - single big DMA per logical tensor (host pre-tiles into the SBUF
    layout), ordered so the first matmul can start ~2us in.
  - out projection runs between attention halves so the 8MB f32 output
    DMA drains during compute.
"""

import sys
import types

sys.path.insert(0, "/opt/trn_rl_repo")

import numpy as np
import ml_dtypes

BF16 = ml_dtypes.bfloat16
F8E4 = ml_dtypes.float8_e4m3
F8E5 = ml_dtypes.float8_e5m2

B, S, D, H, HD = 4, 2048, 1024, 8, 128
HG = 2           # head groups
HPG = H // HG    # heads per group (4)
GD = HPG * HD    # dims per group (512)
SCALE = float(HD) ** -0.5
NKC = S // 128   # 16 k chunks
NSC = S // 128   # 16 s chunks
NP = 4           # d-pairs for DoubleRow (4 x 2 x 128 = 1024)
WSCALE = 64.0    # fp8 weight pre-scale

_cached = {}


def _install_ntff_hook_shim():
    if "antenv.axon_hooks" in sys.modules:
        return
    try:
        import trn_agent_boot.trn_boot as _tb

        _hook = _tb._ntff_profile_via_ctypes("/opt/axon/libaxon_pjrt.so")
    except Exception:
        _hook = None
    _m = types.ModuleType("antenv.axon_hooks")
    _m.get_axon_ntff_profile_hook = lambda: _hook
    sys.modules["antenv.axon_hooks"] = _m


def _split_waits(nc, mybir, maxw=1):
    """Walrus in this image allows only one sync wait per instruction;
    hoist extra waits onto preceding NoOps on the same engine."""
    n_new = 0
    for fn in nc.m.functions:
        for bb in fn.blocks:
            newlist = []
            for inst in bb.instructions:
                si = inst.sync_info
                if si is not None and si.on_wait is not None and len(si.on_wait) > maxw:
                    waits = list(si.on_wait)
                    extra, keep = waits[:-maxw], waits[-maxw:]
                    while extra:
                        chunk, extra = extra[:maxw], extra[maxw:]
                        nop = mybir.InstNoOp(name=f"I-waitsplit-{nc.next_id()}")
                        nop.engine = inst.engine
                        nop.sync_info = mybir.SyncInfo(on_wait=chunk, on_update=[])
                        newlist.append(nop)
                        n_new += 1
                    si.on_wait = keep
                newlist.append(inst)
            bb.instructions = newlist
    return n_new


def _build_program(split_waits=True):
    import contextlib

    import concourse.bass as bass
    import concourse.mybir as mybir
    import concourse.tile as tile

    f32 = mybir.dt.float32
    bf16 = mybir.dt.bfloat16
    f8e4 = mybir.dt.float8e4
    f8e5 = mybir.dt.float8e5
    DR = mybir.MatmulPerfMode.DoubleRow
    Exp = mybir.ActivationFunctionType.Exp
    Ident = mybir.ActivationFunctionType.Identity
    Ln = mybir.ActivationFunctionType.Ln

    nc = bass.Bass()

    # DRAM parameters (per-core shards, pre-tiled on host into SBUF layouts)
    w8q = [nc.declare_dram_parameter(f"w8q{p}", [128, 2, GD], f8e4, isOutput=False)
           for p in range(NP)]
    w8k = [nc.declare_dram_parameter(f"w8k{p}", [128, 2, GD], f8e4, isOutput=False)
           for p in range(NP)]
    x8 = [nc.declare_dram_parameter(f"x8{p}", [128, 2, S], f8e4, isOutput=False)
          for p in range(NP)]
    biases = nc.declare_dram_parameter("biases", [128, 2 * HPG], f32, isOutput=False)
    xTh = [nc.declare_dram_parameter(f"xT{hf}", [128, 4, S], bf16, isOutput=False)
           for hf in range(2)]
    wv = nc.declare_dram_parameter("wv", [128, 8, GD], bf16, isOutput=False)
    outw = nc.declare_dram_parameter("outw", [128, HPG, D], bf16, isOutput=False)
    mask8 = [nc.declare_dram_parameter(f"mask8{qu}", [128, NKC // 4, S], f8e4,
                                       isOutput=False) for qu in range(4)]
    out = nc.declare_dram_parameter("out", [S, D], f32, isOutput=True)

    with tile.TileContext(nc) as tc:
        with contextlib.ExitStack() as ctx:
            p_pers = ctx.enter_context(tc.tile_pool(name="pers", bufs=1))
            p_pm = ctx.enter_context(tc.tile_pool(name="pm", bufs=6))
            p_sh8 = ctx.enter_context(tc.tile_pool(name="sh8", bufs=4))
            p_sm = ctx.enter_context(tc.tile_pool(name="small", bufs=2))
            p_ob = ctx.enter_context(tc.tile_pool(name="ob", bufs=2))
            # two PSUM pools of 4 banks each: rotating tiles (sT / proj /
            # outproj) in pp_rot, per-half persistent accumulators in pp_acc
            pp_rot = ctx.enter_context(tc.tile_pool(name="psrot", bufs=2, space="PSUM"))
            pp_acc = ctx.enter_context(tc.tile_pool(name="psacc", bufs=4, space="PSUM"))

            ones128 = p_pers.tile([128, 128], bf16, tag="ones128", name="ones128")
            nc.vector.memset(ones128, 1.0)

            # --- DMA: P1-critical tensors first ---
            w8q_sb, w8k_sb, x8_sb = [], [], []
            for p in range(NP):
                t = p_pers.tile([128, 2, GD], f8e4, tag=f"w8q{p}", name=f"w8q{p}")
                nc.sync.dma_start(out=t, in_=w8q[p][:, :, :])
                w8q_sb.append(t)
                t = p_sh8.tile([128, 2, S], f8e4, tag="sh8", name=f"x8{p}")
                nc.sync.dma_start(out=t, in_=x8[p][:, :, :])
                x8_sb.append(t)
            bias_sb = p_pers.tile([128, 2 * HPG], f32, tag="biases", name="biases")
            nc.sync.dma_start(out=bias_sb, in_=biases[:, :])
            for p in range(NP):
                t = p_pers.tile([128, 2, GD], f8e4, tag=f"w8k{p}", name=f"w8k{p}")
                nc.sync.dma_start(out=t, in_=w8k[p][:, :, :])
                w8k_sb.append(t)
            # v-projection inputs
            wv_sb = p_pers.tile([128, 8, GD], bf16, tag="wv", name="wv")
            nc.sync.dma_start(out=wv_sb, in_=wv[:, :, :])
            xT_sb = []
            for hf in range(2):
                t = p_pers.tile([128, 4, S], bf16, tag=f"xT{hf}", name=f"xT{hf}")
                nc.sync.dma_start(out=t, in_=xTh[hf][:, :, :])
                xT_sb.append(t)
            outw_sb = p_pers.tile([128, HPG, D], bf16, tag="outw", name="outw")
            nc.sync.dma_start(out=outw_sb, in_=outw[:, :, :])

            def mask_sl(kc, q0, q1):
                return mask_sb[kc // 4][:, kc % 4, q0:q1]

            # --- P1a: q/k projections, fp8 DoubleRow (K=2x128/instr), bf16 out
            qT_sb = [p_pers.tile([128, S], bf16, tag=f"qT{h}", name=f"qT{h}")
                     for h in range(HPG)]
            kT_sb = [p_pers.tile([128, S], bf16, tag=f"kT{h}", name=f"kT{h}")
                     for h in range(HPG)]
            for st, (wsb, dst) in enumerate(((w8q_sb, qT_sb), (w8k_sb, kT_sb))):
                for h in range(HPG):
                    pss = [pp_acc.tile([128, 512], f32, tag="ps", name="ps")
                           for _ in range(4)]
                    for p in range(NP):
                        for qu in range(4):
                            nc.tensor.matmul(
                                pss[qu],
                                lhsT=wsb[p][:, :, h * 128:(h + 1) * 128],
                                rhs=x8_sb[p][:, :, qu * 512:(qu + 1) * 512],
                                start=(p == 0),
                                stop=(p == NP - 1),
                                perf_mode=DR,
                            )
                    for qu in range(4):
                        nc.scalar.activation(
                            out=dst[h][:, qu * 512:(qu + 1) * 512],
                            in_=pss[qu],
                            func=Ident,
                            scale=1.0 / WSCALE,
                            bias=bias_sb[:, st * HPG + h:st * HPG + h + 1],
                        )

            # --- mask load: reuses the x8 slots (x8 dead after P1a) ---
            mask_sb = []
            for qu in range(4):
                t = p_sh8.tile([128, NKC // 4, S], f8e4, tag="sh8", name=f"m{qu}")
                nc.sync.dma_start(out=t, in_=mask8[qu][:, :, :])
                mask_sb.append(t)

            # --- P1b: v projection, bf16 (bias folded into host output) ---
            v_sb = [p_pers.tile([128, GD], bf16, tag=f"v{sc}", name=f"v{sc}")
                    for sc in range(NSC)]
            for g4 in range(0, NSC, 4):
                pss = [pp_acc.tile([128, GD], f32, tag="ps", name="ps") for _ in range(4)]
                for d in range(8):
                    for i, ps in enumerate(pss):
                        sc = g4 + i
                        nc.tensor.matmul(
                            ps,
                            lhsT=xT_sb[d // 4][:, d % 4, sc * 128:(sc + 1) * 128],
                            rhs=wv_sb[:, d, :],
                            start=(d == 0),
                            stop=(d == 7),
                        )
                for i, ps in enumerate(pss):
                    nc.vector.tensor_copy(v_sb[g4 + i], ps)

            # --- P2/P3: attention halves interleaved with out-projection ---
            oT_sb = [p_pers.tile([128, S], bf16, tag=f"oT{h}", name=f"oT{h}")
                     for h in range(HPG)]

            def attention(h, half):
                q0 = half * 1024
                o_ps = [pp_acc.tile([128, 512], f32, tag="ps", name="ps") for _ in range(2)]
                d_ps = [pp_acc.tile([128, 512], f32, tag="ps", name="ps") for _ in range(2)]

                def consume(kc, pm):
                    for qq in range(2):
                        nc.tensor.matmul(
                            o_ps[qq],
                            lhsT=v_sb[kc][:, h * 128:(h + 1) * 128],
                            rhs=pm[:, qq * 512:(qq + 1) * 512],
                            start=(kc == 0),
                            stop=(kc == NKC - 1),
                        )
                    for qq in range(2):
                        nc.tensor.matmul(
                            d_ps[qq],
                            lhsT=ones128,
                            rhs=pm[:, qq * 512:(qq + 1) * 512],
                            start=(kc == 0),
                            stop=(kc == NKC - 1),
                        )

                pending = []
                for kc in range(NKC):
                    sT = pp_rot.tile([128, 1024], f32, tag="st", name="st")
                    for nn in range(2):
                        nc.tensor.matmul(
                            sT[:, nn * 512:(nn + 1) * 512],
                            lhsT=kT_sb[h][:, kc * 128:(kc + 1) * 128],
                            rhs=qT_sb[h][:, q0 + nn * 512:q0 + (nn + 1) * 512],
                            start=True,
                            stop=True,
                        )
                    p = p_pm.tile([128, 1024], bf16, tag="p", name="p")
                    nc.scalar.activation(out=p, in_=sT, func=Exp, scale=SCALE)
                    pm = p_pm.tile([128, 1024], bf16, tag="pm", name="pm")
                    nc.vector.tensor_mul(pm, p, mask_sl(kc, q0, q0 + 1024))
                    pending.append((kc, pm))
                    if len(pending) > 2:
                        consume(*pending.pop(0))
                for item in pending:
                    consume(*item)
                for qq in range(2):
                    lnd = p_sm.tile([128, 512], f32, tag="lnd", name="lnd")
                    nc.scalar.activation(out=lnd, in_=d_ps[qq], func=Ln)
                    rdb = p_sm.tile([128, 512], f32, tag="rdb", name="rdb")
                    nc.scalar.activation(out=rdb, in_=lnd, func=Exp, scale=-1.0)
                    nc.vector.tensor_mul(
                        oT_sb[h][:, q0 + qq * 512:q0 + (qq + 1) * 512],
                        o_ps[qq],
                        rdb,
                    )

            def outproj(sc):
                ps = pp_rot.tile([128, 1024], f32, tag="st", name="st")
                for h in range(HPG):
                    for nn in range(2):
                        nc.tensor.matmul(
                            ps[:, nn * 512:(nn + 1) * 512],
                            lhsT=oT_sb[h][:, sc * 128:(sc + 1) * 128],
                            rhs=outw_sb[:, h, nn * 512:(nn + 1) * 512],
                            start=(h == 0),
                            stop=(h == HPG - 1),
                        )
                osb = p_ob.tile([128, 1024], f32, tag="osb", name="osb")
                nc.vector.tensor_copy(osb, ps)
                nc.sync.dma_start(out=out[sc * 128:(sc + 1) * 128, :], in_=osb)

            for half in range(2):
                for h in range(HPG):
                    attention(h, half)
                for sc in range(half * 8, half * 8 + 8):
                    outproj(sc)

    if split_waits:
        _split_waits(nc, mybir, maxw=1)
    return nc


def _prep_core_inputs(x, attn_mask, qkv_w, qkv_b, q_w, q_b, k_w, k_b, v_w, v_b,
                      out_w):
    """Host-side: fold projections, shard, pre-transpose/tile, cast."""
    f = np.float32
    x = np.asarray(x, f)
    qkv_w = np.asarray(qkv_w, f)
    qkv_b = np.asarray(qkv_b, f)
    Ws = {}
    bs = {}
    for i, (w, b) in enumerate(((q_w, q_b), (k_w, k_b), (v_w, v_b))):
        w = np.asarray(w, f)
        b = np.asarray(b, f)
        sl = slice(i * D, (i + 1) * D)
        Ws[i] = w @ qkv_w[sl]              # [D, D] effective
        bs[i] = b + w @ qkv_b[sl]          # [D]
    out_wT = np.ascontiguousarray(np.asarray(out_w, f).T)  # [D(hd), D(model)]

    # multiplicative keep mask, transposed [k, q], {1, 0} fp8e4, [128,4,S] x4
    keepT = np.where(np.asarray(attn_mask).T == 1, 0.0, 1.0).astype(F8E4)
    mask_t = [np.ascontiguousarray(
        keepT.reshape(4, 4, 128, S)[qu].transpose(1, 0, 2)) for qu in range(4)]

    xT_all, x8_all = [], []
    for b_i in range(B):
        xT = np.ascontiguousarray(x[b_i].T)                # [D, S] f32
        # bf16 path: [2, 128, 4, S]
        xb = xT.astype(BF16).reshape(2, 4, 128, S).transpose(0, 2, 1, 3)
        xT_all.append([np.ascontiguousarray(xb[hf]) for hf in range(2)])
        # fp8 path: [4, 128, 2, S]
        x8 = xT.astype(F8E4).reshape(NP, 2, 128, S).transpose(0, 2, 1, 3)
        x8_all.append([np.ascontiguousarray(x8[p]) for p in range(NP)])

    maps = []
    for c in range(8):
        b_i = c % B
        g = c // B
        sl = slice(g * GD, (g + 1) * GD)
        # fp8 q/k weights: [D, GD] -> [NP, 128, 2, GD], x64
        w8 = {}
        for i, nm in ((0, "q"), (1, "k")):
            wT = np.ascontiguousarray(Ws[i][sl].T) * WSCALE  # [D, GD]
            w8[nm] = wT.astype(F8E4).reshape(NP, 2, 128, GD).transpose(0, 2, 1, 3)
        # biases [128, 2*HPG]: q heads then k heads, per-head [128]
        bias = np.concatenate([
            bs[0][sl].reshape(HPG, 128).T, bs[1][sl].reshape(HPG, 128).T,
        ], axis=1).astype(f)
        wv_t = np.ascontiguousarray(
            Ws[2][sl].T.astype(BF16).reshape(8, 128, GD).transpose(1, 0, 2))
        outw_t = np.ascontiguousarray(
            out_wT[sl].astype(BF16).reshape(HPG, 128, D).transpose(1, 0, 2))
        m = {
            "biases": np.ascontiguousarray(bias),
            "wv": wv_t,
            "outw": outw_t,
        }
        for p in range(NP):
            m[f"w8q{p}"] = np.ascontiguousarray(w8["q"][p])
            m[f"w8k{p}"] = np.ascontiguousarray(w8["k"][p])
            m[f"x8{p}"] = x8_all[b_i][p]
        for hf in range(2):
            m[f"xT{hf}"] = xT_all[b_i][hf]
        for qu in range(4):
            m[f"mask8{qu}"] = mask_t[qu]
        maps.append(m)
    return maps


def kernel(x, attn_mask, qkv_w, qkv_b, q_w, q_b, k_w, k_b, v_w, v_b,
           out_w, out_b, _trace=False):
    _install_ntff_hook_shim()
    from concourse.bass_utils import run_bass_kernel_spmd

    in_maps = _prep_core_inputs(
        x, attn_mask, qkv_w, qkv_b, q_w, q_b, k_w, k_b, v_w, v_b, out_w
    )
    if "nc" not in _cached:
        _cached["nc"] = _build_program()
    nc = _cached["nc"]
    core_ids = list(range(8))
    try:
        res = run_bass_kernel_spmd(nc, in_maps, core_ids, trace=_trace)
    except Exception:
        res = run_bass_kernel_spmd(nc, in_maps, core_ids, trace=_trace)
    _cached["last_result"] = res

    # host-side bias: out_b plus the v-bias pushed through attention+outproj
    f = np.float32
    qkv_b_ = np.asarray(qkv_b, f)
    bv = np.asarray(v_b, f) + np.asarray(v_w, f) @ qkv_b_[2 * D:]
    out_wT = np.asarray(out_w, f).T
    bias_full = np.asarray(out_b, f) + bv @ out_wT

    full = np.empty((B, S, D), np.float32)
    for b_i in range(B):
        full[b_i] = (
            res.results[b_i]["out"] + res.results[b_i + B]["out"] + bias_full
        )
    return full


# revision 3
# speedup vs baseline: 1.0507x; 1.0507x over previous
"""Multi-head self-attention Trainium2 kernel (8 NeuronCores), v2.

Problem: B=4, S=2048, D=1024, H=8 heads (HD=128).
  qkv = x @ qkv_w.T + qkv_b ; q,k,v = split(qkv)
  q = (q @ q_w.T + q_b)  (same k, v) -> [B,H,S,HD]
  scores = q k^T * HD^-0.5, masked softmax (attn_mask==1 -> -inf), o = attn @ v
  out = o @ out_w.T + out_b

Sharding: 8 cores = 4 batches x 2 head-groups (4 heads each).
Core c: batch b = c % 4, head-group g = c // 4.

Host-side algebraic folding: per-stream projections composed with the qkv
projection into single effective weights; the v bias passes through softmax
unchanged (weights sum to 1) so it folds into the host-side output bias.

v3 speed structure vs v1:
  - q/k projections run in fp8e4 DoubleRow mode (K=2x128 per instruction,
    0.5 cyc/row): weights are scaled x64 on host to stay in fp8 normal
    range; the ACT copy out of PSUM rescales by 1/64 and adds the bias,
    writing fp8 directly.
  - scores stay bf16: fp8 DoubleRow with K=2x64 gives no HW speedup (DR
    only halves instruction count when K=256; stream rate is unchanged),
    and DVE/ACT ops touching PSUM at 512-col granularity halved the
    attention-phase matmul cadence in v2 — the attention inner loop is
    the v1 shape (sT [128,1024] PSUM -> ACT exp -> DVE mask-mul in SBUF),
    with the keep mask stored fp8e4 {1,0} (4MB DMA instead of 8MB).
  - v / o / out-projection matmuls stay bf16: fp8 there provably exceeds
    the error budget (random-sign dot products keep elementwise rel err).
  - single big DMA per logical tensor (host pre-tiles into the SBUF
    layout), ordered so the first matmul can start ~2us in.
  - out projection runs between attention halves so the 8MB f32 output
    DMA drains during compute.
"""

import sys
import types

sys.path.insert(0, "/opt/trn_rl_repo")

import numpy as np
import ml_dtypes

BF16 = ml_dtypes.bfloat16
F8E4 = ml_dtypes.float8_e4m3
F8E5 = ml_dtypes.float8_e5m2

B, S, D, H, HD = 4, 2048, 1024, 8, 128
HG = 2           # head groups
HPG = H // HG    # heads per group (4)
GD = HPG * HD    # dims per group (512)
SCALE = float(HD) ** -0.5
NKC = S // 128   # 16 k chunks
NSC = S // 128   # 16 s chunks
NP = 4           # d-pairs for DoubleRow (4 x 2 x 128 = 1024)
WSCALE = 64.0    # fp8 weight pre-scale

_cached = {}


def _install_ntff_hook_shim():
    if "antenv.axon_hooks" in sys.modules:
        return
    try:
        import trn_agent_boot.trn_boot as _tb

        _hook = _tb._ntff_profile_via_ctypes("/opt/axon/libaxon_pjrt.so")
    except Exception:
        _hook = None
    _m = types.ModuleType("antenv.axon_hooks")
    _m.get_axon_ntff_profile_hook = lambda: _hook
    sys.modules["antenv.axon_hooks"] = _m


def _split_waits(nc, mybir, maxw=1):
    """Walrus in this image allows only one sync wait per instruction;
    hoist extra waits onto preceding NoOps on the same engine."""
    n_new = 0
    for fn in nc.m.functions:
        for bb in fn.blocks:
            newlist = []
            for inst in bb.instructions:
                si = inst.sync_info
                if si is not None and si.on_wait is not None and len(si.on_wait) > maxw:
                    waits = list(si.on_wait)
                    extra, keep = waits[:-maxw], waits[-maxw:]
                    while extra:
                        chunk, extra = extra[:maxw], extra[maxw:]
                        nop = mybir.InstNoOp(name=f"I-waitsplit-{nc.next_id()}")
                        nop.engine = inst.engine
                        nop.sync_info = mybir.SyncInfo(on_wait=chunk, on_update=[])
                        newlist.append(nop)
                        n_new += 1
                    si.on_wait = keep
                newlist.append(inst)
            bb.instructions = newlist
    return n_new


def _build_program(split_waits=True):
    import contextlib

    import concourse.bass as bass
    import concourse.mybir as mybir
    import concourse.tile as tile

    f32 = mybir.dt.float32
    bf16 = mybir.dt.bfloat16
    f8e4 = mybir.dt.float8e4
    f8e5 = mybir.dt.float8e5
    DR = mybir.MatmulPerfMode.DoubleRow
    Exp = mybir.ActivationFunctionType.Exp
    Ident = mybir.ActivationFunctionType.Identity
    Ln = mybir.ActivationFunctionType.Ln

    nc = bass.Bass()

    # DRAM parameters (per-core shards, pre-tiled on host into SBUF layouts)
    w8q = [nc.declare_dram_parameter(f"w8q{p}", [128, 2, GD], f8e4, isOutput=False)
           for p in range(NP)]
    w8k = [nc.declare_dram_parameter(f"w8k{p}", [128, 2, GD], f8e4, isOutput=False)
           for p in range(NP)]
    x8 = [nc.declare_dram_parameter(f"x8{p}{hb}", [128, 2, S // 2], f8e4,
                                    isOutput=False)
          for p in range(NP) for hb in range(2)]
    biases = nc.declare_dram_parameter("biases", [128, 2 * HPG], f32, isOutput=False)
    xTd = [nc.declare_dram_parameter(f"xT{d}", [128, S], bf16, isOutput=False)
           for d in range(8)]
    wv = nc.declare_dram_parameter("wv", [128, 8, GD], bf16, isOutput=False)
    outw = nc.declare_dram_parameter("outw", [128, HPG, D], bf16, isOutput=False)
    maskd = [nc.declare_dram_parameter(f"mask{kc}", [128, S], bf16,
                                       isOutput=False) for kc in range(NKC)]
    out = nc.declare_dram_parameter("out", [S, D], f32, isOutput=True)

    with tile.TileContext(nc) as tc:
        with contextlib.ExitStack() as ctx:
            p_pers = ctx.enter_context(tc.tile_pool(name="pers", bufs=1))
            p_pm = ctx.enter_context(tc.tile_pool(name="pm", bufs=6))
            p_sh = ctx.enter_context(tc.tile_pool(name="sh", bufs=16))
            p_sm = ctx.enter_context(tc.tile_pool(name="small", bufs=2))
            p_ob = ctx.enter_context(tc.tile_pool(name="ob", bufs=3))
            # two PSUM pools of 4 banks each: rotating tiles (sT / proj /
            # outproj) in pp_rot, per-half persistent accumulators in pp_acc
            pp_rot = ctx.enter_context(tc.tile_pool(name="psrot", bufs=2, space="PSUM"))
            pp_acc = ctx.enter_context(tc.tile_pool(name="psacc", bufs=4, space="PSUM"))

            ones128 = p_pers.tile([128, 128], bf16, tag="ones128", name="ones128")
            nc.vector.memset(ones128, 1.0)

            # --- DMA: P1-critical tensors first ---
            w8q_sb, w8k_sb, x8_sb = [], [], []
            for p in range(NP):
                t = p_pers.tile([128, 2, GD], f8e4, tag=f"w8q{p}", name=f"w8q{p}")
                nc.sync.dma_start(out=t, in_=w8q[p][:, :, :])
                w8q_sb.append(t)
                halves = []
                for hb in range(2):
                    t = p_sh.tile([128, 2, S // 2], f8e4, tag="sh", name=f"x8{p}{hb}")
                    nc.sync.dma_start(out=t, in_=x8[2 * p + hb][:, :, :])
                    halves.append(t)
                x8_sb.append(halves)
            bias_sb = p_pers.tile([128, 2 * HPG], f32, tag="biases", name="biases")
            nc.sync.dma_start(out=bias_sb, in_=biases[:, :])
            for p in range(NP):
                t = p_pers.tile([128, 2, GD], f8e4, tag=f"w8k{p}", name=f"w8k{p}")
                nc.sync.dma_start(out=t, in_=w8k[p][:, :, :])
                w8k_sb.append(t)
            # v-projection inputs
            wv_sb = p_pers.tile([128, 8, GD], bf16, tag="wv", name="wv")
            nc.sync.dma_start(out=wv_sb, in_=wv[:, :, :])
            xT_sb = []
            for dd in range(8):
                t = p_sh.tile([128, S], bf16, tag="sh", name=f"xT{dd}")
                nc.sync.dma_start(out=t, in_=xTd[dd][:, :])
                xT_sb.append(t)
            outw_sb = p_pers.tile([128, HPG, D], bf16, tag="outw", name="outw")
            nc.sync.dma_start(out=outw_sb, in_=outw[:, :, :])

            def mask_sl(kc, q0, q1):
                return mask_sb[kc][:, q0:q1]

            # --- P1a: q/k projections, fp8 DoubleRow (K=2x128/instr), bf16 out
            qT_sb = [p_pers.tile([128, S], bf16, tag=f"qT{h}", name=f"qT{h}")
                     for h in range(HPG)]
            kT_sb = [p_pers.tile([128, S], bf16, tag=f"kT{h}", name=f"kT{h}")
                     for h in range(HPG)]
            for st, (wsb, dst) in enumerate(((w8q_sb, qT_sb), (w8k_sb, kT_sb))):
                for h in range(HPG):
                    pss = [pp_acc.tile([128, 512], f32, tag="ps", name="ps")
                           for _ in range(4)]
                    for p in range(NP):
                        for qu in range(4):
                            nc.tensor.matmul(
                                pss[qu],
                                lhsT=wsb[p][:, :, h * 128:(h + 1) * 128],
                                rhs=x8_sb[p][qu // 2][:, :, (qu % 2) * 512:(qu % 2 + 1) * 512],
                                start=(p == 0),
                                stop=(p == NP - 1),
                                perf_mode=DR,
                            )
                    for qu in range(4):
                        nc.scalar.activation(
                            out=dst[h][:, qu * 512:(qu + 1) * 512],
                            in_=pss[qu],
                            func=Ident,
                            scale=1.0 / WSCALE,
                            bias=bias_sb[:, st * HPG + h:st * HPG + h + 1],
                        )

            # --- P1b: v projection, bf16 (bias folded into host output) ---
            v_sb = [p_pers.tile([128, GD], bf16, tag=f"v{sc}", name=f"v{sc}")
                    for sc in range(NSC)]
            for g4 in range(0, NSC, 4):
                pss = [pp_acc.tile([128, GD], f32, tag="ps", name="ps") for _ in range(4)]
                for d in range(8):
                    for i, ps in enumerate(pss):
                        sc = g4 + i
                        nc.tensor.matmul(
                            ps,
                            lhsT=xT_sb[d][:, sc * 128:(sc + 1) * 128],
                            rhs=wv_sb[:, d, :],
                            start=(d == 0),
                            stop=(d == 7),
                        )
                for i, ps in enumerate(pss):
                    nc.vector.tensor_copy(v_sb[g4 + i], ps)

            # --- mask load: rotates onto x8/xT slots as P1 frees them ---
            mask_sb = []
            for kc in range(NKC):
                t = p_sh.tile([128, S], bf16, tag="sh", name=f"m{kc}")
                nc.sync.dma_start(out=t, in_=maskd[kc][:, :])
                mask_sb.append(t)

            # --- P2/P3: attention halves interleaved with out-projection ---
            oT_sb = [p_pers.tile([128, S], bf16, tag=f"oT{h}", name=f"oT{h}")
                     for h in range(HPG)]

            def attention(h, half):
                q0 = half * 1024
                o_ps = [pp_acc.tile([128, 512], f32, tag="ps", name="ps") for _ in range(2)]
                d_ps = [pp_acc.tile([128, 512], f32, tag="ps", name="ps") for _ in range(2)]

                def consume(kc, pm):
                    for qq in range(2):
                        nc.tensor.matmul(
                            o_ps[qq],
                            lhsT=v_sb[kc][:, h * 128:(h + 1) * 128],
                            rhs=pm[:, qq * 512:(qq + 1) * 512],
                            start=(kc == 0),
                            stop=(kc == NKC - 1),
                        )
                    for qq in range(2):
                        nc.tensor.matmul(
                            d_ps[qq],
                            lhsT=ones128,
                            rhs=pm[:, qq * 512:(qq + 1) * 512],
                            start=(kc == 0),
                            stop=(kc == NKC - 1),
                        )

                pending = []
                for kc in range(NKC):
                    sT = pp_rot.tile([128, 1024], f32, tag="st", name="st")
                    for nn in range(2):
                        nc.tensor.matmul(
                            sT[:, nn * 512:(nn + 1) * 512],
                            lhsT=kT_sb[h][:, kc * 128:(kc + 1) * 128],
                            rhs=qT_sb[h][:, q0 + nn * 512:q0 + (nn + 1) * 512],
                            start=True,
                            stop=True,
                        )
                    p = p_pm.tile([128, 1024], bf16, tag="p", name="p")
                    nc.scalar.activation(out=p, in_=sT, func=Exp, scale=SCALE)
                    pm = p_pm.tile([128, 1024], bf16, tag="pm", name="pm")
                    nc.vector.tensor_mul(pm, p, mask_sl(kc, q0, q0 + 1024))
                    pending.append((kc, pm))
                    if len(pending) > 2:
                        consume(*pending.pop(0))
                for item in pending:
                    consume(*item)
                for qq in range(2):
                    lnd = p_sm.tile([128, 512], f32, tag="lnd", name="lnd")
                    nc.scalar.activation(out=lnd, in_=d_ps[qq], func=Ln)
                    rdb = p_sm.tile([128, 512], f32, tag="rdb", name="rdb")
                    nc.scalar.activation(out=rdb, in_=lnd, func=Exp, scale=-1.0)
                    nc.vector.tensor_mul(
                        oT_sb[h][:, q0 + qq * 512:q0 + (qq + 1) * 512],
                        o_ps[qq],
                        rdb,
                    )

            def outproj(sc):
                if sc % 2 == 0:
                    ps = pp_rot.tile([128, 1024], f32, tag="st", name="st")
                    pss = [ps[:, 0:512], ps[:, 512:1024]]
                else:
                    pss = [pp_acc.tile([128, 512], f32, tag="ps", name="ps")
                           for _ in range(2)]
                for h in range(HPG):
                    for nn in range(2):
                        nc.tensor.matmul(
                            pss[nn],
                            lhsT=oT_sb[h][:, sc * 128:(sc + 1) * 128],
                            rhs=outw_sb[:, h, nn * 512:(nn + 1) * 512],
                            start=(h == 0),
                            stop=(h == HPG - 1),
                        )
                osb = p_ob.tile([128, 1024], f32, tag="osb", name="osb")
                nc.scalar.activation(out=osb[:, 0:512], in_=pss[0],
                                     func=Ident, scale=1.0)
                nc.vector.tensor_copy(osb[:, 512:1024], pss[1])
                nc.sync.dma_start(out=out[sc * 128:(sc + 1) * 128, :], in_=osb)

            for half in range(2):
                for h in range(HPG):
                    attention(h, half)
                for sc in range(half * 8, half * 8 + 8):
                    outproj(sc)

    if split_waits:
        _split_waits(nc, mybir, maxw=1)
    return nc


def _prep_core_inputs(x, attn_mask, qkv_w, qkv_b, q_w, q_b, k_w, k_b, v_w, v_b,
                      out_w):
    """Host-side: fold projections, shard, pre-transpose/tile, cast."""
    f = np.float32
    x = np.asarray(x, f)
    qkv_w = np.asarray(qkv_w, f)
    qkv_b = np.asarray(qkv_b, f)
    Ws = {}
    bs = {}
    for i, (w, b) in enumerate(((q_w, q_b), (k_w, k_b), (v_w, v_b))):
        w = np.asarray(w, f)
        b = np.asarray(b, f)
        sl = slice(i * D, (i + 1) * D)
        Ws[i] = w @ qkv_w[sl]              # [D, D] effective
        bs[i] = b + w @ qkv_b[sl]          # [D]
    out_wT = np.ascontiguousarray(np.asarray(out_w, f).T)  # [D(hd), D(model)]

    # multiplicative keep mask, transposed [k, q], {1, 0} bf16, [128,S] x16
    keepT = np.where(np.asarray(attn_mask).T == 1, 0.0, 1.0).astype(BF16)
    mask_t = [np.ascontiguousarray(keepT.reshape(NKC, 128, S)[kc])
              for kc in range(NKC)]

    xT_all, x8_all = [], []
    for b_i in range(B):
        xT = np.ascontiguousarray(x[b_i].T)                # [D, S] f32
        # bf16 path: 8 x [128, S]
        xb = xT.astype(BF16).reshape(8, 128, S)
        xT_all.append([np.ascontiguousarray(xb[dd]) for dd in range(8)])
        # fp8 path: [4, 128, 2, S] split into S/2 halves
        x8 = xT.astype(F8E4).reshape(NP, 2, 128, S).transpose(0, 2, 1, 3)
        x8_all.append([np.ascontiguousarray(x8[p][:, :, hb * (S // 2):(hb + 1) * (S // 2)])
                       for p in range(NP) for hb in range(2)])

    maps = []
    for c in range(8):
        b_i = c % B
        g = c // B
        sl = slice(g * GD, (g + 1) * GD)
        # fp8 q/k weights: [D, GD] -> [NP, 128, 2, GD], x64
        w8 = {}
        for i, nm in ((0, "q"), (1, "k")):
            wT = np.ascontiguousarray(Ws[i][sl].T) * WSCALE  # [D, GD]
            w8[nm] = wT.astype(F8E4).reshape(NP, 2, 128, GD).transpose(0, 2, 1, 3)
        # biases [128, 2*HPG]: q heads then k heads, per-head [128]
        bias = np.concatenate([
            bs[0][sl].reshape(HPG, 128).T, bs[1][sl].reshape(HPG, 128).T,
        ], axis=1).astype(f)
        wv_t = np.ascontiguousarray(
            Ws[2][sl].T.astype(BF16).reshape(8, 128, GD).transpose(1, 0, 2))
        outw_t = np.ascontiguousarray(
            out_wT[sl].astype(BF16).reshape(HPG, 128, D).transpose(1, 0, 2))
        m = {
            "biases": np.ascontiguousarray(bias),
            "wv": wv_t,
            "outw": outw_t,
        }
        for p in range(NP):
            m[f"w8q{p}"] = np.ascontiguousarray(w8["q"][p])
            m[f"w8k{p}"] = np.ascontiguousarray(w8["k"][p])
            for hb in range(2):
                m[f"x8{p}{hb}"] = x8_all[b_i][2 * p + hb]
        for dd in range(8):
            m[f"xT{dd}"] = xT_all[b_i][dd]
        for kc in range(NKC):
            m[f"mask{kc}"] = mask_t[kc]
        maps.append(m)
    return maps


def kernel(x, attn_mask, qkv_w, qkv_b, q_w, q_b, k_w, k_b, v_w, v_b,
           out_w, out_b, _trace=False):
    _install_ntff_hook_shim()
    from concourse.bass_utils import run_bass_kernel_spmd

    in_maps = _prep_core_inputs(
        x, attn_mask, qkv_w, qkv_b, q_w, q_b, k_w, k_b, v_w, v_b, out_w
    )
    if "nc" not in _cached:
        _cached["nc"] = _build_program()
    nc = _cached["nc"]
    core_ids = list(range(8))
    try:
        res = run_bass_kernel_spmd(nc, in_maps, core_ids, trace=_trace)
    except Exception:
        res = run_bass_kernel_spmd(nc, in_maps, core_ids, trace=_trace)
    _cached["last_result"] = res

    # host-side bias: out_b plus the v-bias pushed through attention+outproj
    f = np.float32
    qkv_b_ = np.asarray(qkv_b, f)
    bv = np.asarray(v_b, f) + np.asarray(v_w, f) @ qkv_b_[2 * D:]
    out_wT = np.asarray(out_w, f).T
    bias_full = np.asarray(out_b, f) + bv @ out_wT

    full = np.empty((B, S, D), np.float32)
    for b_i in range(B):
        full[b_i] = (
            res.results[b_i]["out"] + res.results[b_i + B]["out"] + bias_full
        )
    return full
